# revision 35
# baseline (speedup 1.0000x reference)
"""Trainium2 Bass kernel for a linear-attention transformer block.

B=8, S=4096, E=512, NH=8, DH=64, HID=2048.
Sharding: data-parallel over batch — one batch element per NeuronCore.
The axon tunnel (~40MB/s, half-duplex, large per-operand overhead)
dominates wall time, so bytes AND operand count are minimized — each
core receives ONE uint8 blob:
  - x int8-quantized on host (scale 32, RNE; x~N(0,1) so the added
    output error is ~9.5e-3), dequantized to bf16 on device;
  - weight shards (1/8 of [wq;wk;wv;wo;w2] and of w1, bf16 bytes),
    AllGathered on-device (HBM-to-HBM collectives via DRAM bounce
    buffers), loaded to SBUF through bitcast DMAs;
  - an aux page holding identity/selectors plus bk/bv as columns and
    the f32 param table split into bf16 hi+lo halves; the ones row,
    head-expand matrix, bk/bv rows and f32 params are reconstructed
    on device (memset / PE transpose / vector add).
The output is int8 (scale 28; post-LN rows are ~unit-variance), host
upcasts to f32; the donated zero output buffer shrinks equally.
Total rel err ~1.49e-2 (gate 2e-2), deterministic. A persistent JAX
compilation cache removes the per-call XLA recompile that
run_bass_kernel_spmd's fresh-closure jit otherwise incurs. Steady-state
matches an I/O-identical no-compute kernel: transfer-floor bound.

Per-core pipeline (feature-major activations, bf16 matmuls, f32 PSUM):
  phase A: x -> xT (PE transpose); qT = elu(Wq^T xT + bq)+1 stored; K,V
           token-major; KVT[d,m] and Ksum accumulated in PSUM over all S.
  phase B: Z = 1/(Q.Ksum+eps); attnT = blockdiag(KVT) @ (Q*Z); Wo; LN1
           (stats via ones-matmuls); FFN; LN2; PE-transpose out.
"""

import numpy as np
import ml_dtypes

from concourse import bass, bacc, tile, mybir
from concourse.bass_utils import run_bass_kernel_spmd

BF16 = ml_dtypes.bfloat16
F32 = np.float32

B, S, E, NH, HID, DH = 8, 4096, 512, 8, 2048, 64
ATTN_EPS = 1e-6
LN_EPS = 1e-5

NCORES = 8
OSCALE = 28.0             # int8 output quantization scale
XSCALE = 32.0             # int8 input quantization scale
WAOFF, WBOFF, AUXOFF = 4096, 5120, 5632   # blob row offsets
BLOB_ROWS = 5760
TT = 512                  # tokens per tile
NT = S // TT              # 8 token tiles
NC_E = E // 128           # 4 feature chunks
NC_H = HID // 128         # 16 hidden chunks
NJ = TT // 128            # 4 token sub-tiles per tile

dt = mybir.dt
AF = mybir.ActivationFunctionType
ALU = mybir.AluOpType

_CACHE = {}


def _ln_norm(nc, pbsb, pbbc, opool, hts, ssum, ssq, onesr_s, g_c, be_c, otag):
    """LayerNorm: per-chunk feature-major tiles + sum/sumsq stats psums."""
    inv = 1.0 / E
    mean = pbsb.tile([1, TT], dt.float32, tag="mean")
    nc.vector.tensor_scalar_mul(mean[:], ssum[:], inv)
    msq = pbsb.tile([1, TT], dt.float32, tag="msq")
    nc.vector.tensor_mul(msq[:], mean[:], mean[:])
    var = pbsb.tile([1, TT], dt.float32, tag="var")
    nc.vector.tensor_scalar(out=var[:], in0=ssq[:], scalar1=inv,
                            scalar2=LN_EPS, op0=ALU.mult, op1=ALU.add)
    nc.vector.tensor_sub(var[:], var[:], msq[:])
    rs = pbsb.tile([1, TT], dt.float32, tag="rs")
    nc.vector.reciprocal(rs[:], var[:])
    nc.scalar.activation(rs[:], rs[:], AF.Sqrt)
    mean_b = pbsb.tile([1, TT], dt.bfloat16, tag="meanb")
    nc.scalar.activation(mean_b[:], mean[:], AF.Copy)
    rs_b = pbsb.tile([1, TT], dt.bfloat16, tag="rsb")
    nc.scalar.activation(rs_b[:], rs[:], AF.Copy)
    mb = pbbc.tile([128, TT], dt.float32, tag="bc")
    nc.tensor.matmul(mb[:], onesr_s[0:1, 0:128], mean_b[:],
                     start=True, stop=True)
    rb = pbbc.tile([128, TT], dt.float32, tag="bc")
    nc.tensor.matmul(rb[:], onesr_s[0:1, 0:128], rs_b[:],
                     start=True, stop=True)
    outs = []
    for c in range(len(hts)):
        tmp = pbsb.tile([128, TT], dt.bfloat16, tag="nrm")
        nc.vector.tensor_sub(tmp[:], hts[c][:], mb[:])
        nc.vector.tensor_mul(tmp[:], tmp[:], rb[:])
        o = opool.tile([128, TT], dt.bfloat16, tag=otag)
        nc.scalar.activation(o[:], tmp[:], AF.Identity,
                             bias=be_c(c), scale=g_c(c))
        outs.append(o)
    return outs


def _build():
    nc = bacc.Bacc("TRN2", target_bir_lowering=False, debug=False,
                   num_devices=NCORES)

    def din(name, shape, d):
        return nc.dram_tensor(name, list(shape), d, kind="ExternalInput")

    # Single input operand per core (per-operand tunnel overhead is large):
    # uint8 blob, 512-byte rows.
    #   rows 0:4096      x int8 [4096, 512]
    #   rows 4096:5120   wa shard: 1/8 of [wq;wk;wv;wo;w2] ([512,512] bf16)
    #   rows 5120:5632   wb shard: 1/8 of w1 ([64,2048] bf16)
    #   rows 5632:5760   aux page [128, 256] bf16:
    #     cols 0:128 identity, 128:130 headsel, 130 ones, 131:135 bk,
    #     135:139 bv, 139:183 pp-hi, 183:227 pp-lo
    # pp cols: 0-3 bq, 4-7 bo, 8-23 b1, 24-27 b2, 28-31 g1, 32-35 be1,
    #          36-39 g2, 40-43 be2 (f32 = hi + lo reconstructed on device)
    blob_d = din("blob", (BLOB_ROWS, 512), dt.uint8)
    out_d = nc.dram_tensor("out", [S, E], dt.int8, kind="ExternalOutput")

    with tile.TileContext(nc) as tc:
        from contextlib import ExitStack
        es = ExitStack()
        with es:
            dpool = es.enter_context(
                tc.tile_pool(name="wdram", bufs=1, space="DRAM"))
            cpool = es.enter_context(tc.tile_pool(name="const", bufs=1))

            # ---- AllGather the weight shards into full DRAM copies ----
            # boutA rows are weight rows (1024B each, bitcast to bf16 on
            # the SBUF loads); boutB rows are w1 rows (4096B each).
            rg = [list(range(NCORES))]
            binA = dpool.tile([1024, 512], dt.uint8, tag="biA", name="biA")
            boutA = dpool.tile([4 * E + HID, 1024], dt.uint8,
                               tag="boA", name="boA")
            nc.gpsimd.dma_start(binA[:], blob_d[WAOFF:WBOFF, :])
            nc.gpsimd.collective_compute(
                "AllGather", mybir.AluOpType.bypass, replica_groups=rg,
                ins=[binA.opt()], outs=[boutA.opt()])
            binB = dpool.tile([512, 512], dt.uint8, tag="biB", name="biB")
            boutB = dpool.tile([E, 4096], dt.uint8, tag="boB", name="boB")
            nc.gpsimd.dma_start(binB[:], blob_d[WBOFF:AUXOFF, :])
            nc.gpsimd.collective_compute(
                "AllGather", mybir.AluOpType.bypass, replica_groups=rg,
                ins=[binB.opt()], outs=[boutB.opt()])
            OQ, OK_, OV, OO, O2 = 0, E, 2 * E, 3 * E, 4 * E

            wq_s = cpool.tile([128, NC_E * E], dt.bfloat16, tag="wq")
            wk_s = cpool.tile([128, NC_E * E], dt.bfloat16, tag="wk")
            wv_s = cpool.tile([128, NC_E * E], dt.bfloat16, tag="wv")
            wo_s = cpool.tile([128, NC_E * E], dt.bfloat16, tag="wo")
            w1_s = cpool.tile([128, NC_E * HID], dt.bfloat16, tag="w1")
            w2_s = cpool.tile([128, NC_H * E], dt.bfloat16, tag="w2")
            pp_s = cpool.tile([128, 44], dt.float32, tag="pp")
            aux_s = cpool.tile([128, 256], dt.bfloat16, tag="aux")
            tsb = cpool.tile([11, 128], dt.bfloat16, tag="tsb")
            onesr_s = cpool.tile([1, TT], dt.bfloat16, tag="onesr")
            bk_s = cpool.tile([1, E], dt.bfloat16, tag="bk")
            bv_s = cpool.tile([1, E], dt.bfloat16, tag="bv")
            qt_s = [cpool.tile([128, S], dt.bfloat16, tag=f"qt{c}", name=f"qt{c}")
                    for c in range(NC_E)]
            xt_s = [cpool.tile([128, S], dt.bfloat16, tag=f"xt{c}", name=f"xt{c}")
                    for c in range(NC_E)]
            kvt_s = cpool.tile([128, NC_E * 128], dt.bfloat16, tag="kvt")
            ksumb_s = cpool.tile([1, E], dt.bfloat16, tag="ksumb")
            ksc_s = cpool.tile([128, NC_E], dt.float32, tag="ksc")

            for c in range(NC_E):
                nc.sync.dma_start(
                    out=wq_s[:, c * E:(c + 1) * E],
                    in_=boutA[OQ + c * 128:OQ + (c + 1) * 128, :]
                    .bitcast(dt.bfloat16))
                nc.sync.dma_start(
                    out=wk_s[:, c * E:(c + 1) * E],
                    in_=boutA[OK_ + c * 128:OK_ + (c + 1) * 128, :]
                    .bitcast(dt.bfloat16))
                nc.sync.dma_start(
                    out=wv_s[:, c * E:(c + 1) * E],
                    in_=boutA[OV + c * 128:OV + (c + 1) * 128, :]
                    .bitcast(dt.bfloat16))
                nc.sync.dma_start(
                    out=wo_s[:, c * E:(c + 1) * E],
                    in_=boutA[OO + c * 128:OO + (c + 1) * 128, :]
                    .bitcast(dt.bfloat16))
                nc.sync.dma_start(
                    out=w1_s[:, c * HID:(c + 1) * HID],
                    in_=boutB[c * 128:(c + 1) * 128, :].bitcast(dt.bfloat16))
            for j in range(NC_H):
                nc.sync.dma_start(
                    out=w2_s[:, j * E:(j + 1) * E],
                    in_=boutA[O2 + j * 128:O2 + (j + 1) * 128, :]
                    .bitcast(dt.bfloat16))
            nc.sync.dma_start(out=aux_s[:],
                              in_=blob_d[AUXOFF:AUXOFF + 128, :]
                              .bitcast(dt.bfloat16))

            idb = aux_s[:, 0:128]            # bf16 identity
            hsel = aux_s[:, 128:130]         # [128,2] head select
            onesc = aux_s[:, 130:131]        # [128,1] ones col
            ones1x128 = onesr_s[0:1, 0:128]  # [1,128]

            # reconstruct small params on device (saves tunnel operands):
            # pp f32 = hi + lo halves; ones row via memset; hexp/bk/bv via
            # PE transpose of aux cols 128:139 -> [11,128].
            nc.vector.tensor_add(pp_s[:], aux_s[:, 139:183],
                                 aux_s[:, 183:227])
            nc.vector.memset(onesr_s[:], 1.0)
            with tc.tile_pool(name="init_ps", bufs=1, space="PSUM") as ips:
                tp0 = ips.tile([11, 128], dt.bfloat16, tag="tp0")
                nc.tensor.transpose(tp0[:], aux_s[:, 128:139], idb)
                nc.scalar.activation(tsb[:], tp0[:], AF.Copy)
            for c in range(NC_E):
                nc.sync.dma_start(out=bk_s[0:1, c * 128:(c + 1) * 128],
                                  in_=tsb[3 + c:4 + c, :])
                nc.sync.dma_start(out=bv_s[0:1, c * 128:(c + 1) * 128],
                                  in_=tsb[7 + c:8 + c, :])

            bq_c = lambda c: pp_s[:, c:c + 1]
            bo_c = lambda c: pp_s[:, 4 + c:5 + c]
            b1_c = lambda j: pp_s[:, 8 + j:9 + j]
            b2_c = lambda c: pp_s[:, 24 + c:25 + c]
            g1_c = lambda c: pp_s[:, 28 + c:29 + c]
            be1_c = lambda c: pp_s[:, 32 + c:33 + c]
            g2_c = lambda c: pp_s[:, 36 + c:37 + c]
            be2_c = lambda c: pp_s[:, 40 + c:41 + c]

            # =========================== PHASE A ==========================
            with tc.tile_pool(name="acc_ps", bufs=1, space="PSUM") as accp, \
                 tc.tile_pool(name="pa_ps", bufs=2, space="PSUM") as paps, \
                 tc.tile_pool(name="tp_ps", bufs=2, space="PSUM") as tpps, \
                 tc.tile_pool(name="pa_x", bufs=4, space="SBUF") as pax, \
                 tc.tile_pool(name="pa_t", bufs=2, space="SBUF") as pat, \
                 tc.tile_pool(name="pa_kv", bufs=3, space="SBUF") as pakv:

                kvt_ps = accp.tile([128, NC_E * 128], dt.float32, tag="kvtp")
                ksum_ps = accp.tile([1, E], dt.float32, tag="ksump")

                first_kv = True
                for t in range(NT):
                    t0 = t * TT
                    xtoks = []
                    for j in range(NJ):
                        xq_j = pax.tile([128, E], dt.int8, tag="xtokq")
                        nc.sync.dma_start(
                            out=xq_j[:],
                            in_=blob_d[t0 + j * 128: t0 + (j + 1) * 128, :]
                            .bitcast(dt.int8))
                        xt_j = pax.tile([128, E], dt.bfloat16, tag="xtok")
                        nc.vector.tensor_scalar_mul(xt_j[:], xq_j[:],
                                                    1.0 / XSCALE)
                        xtoks.append(xt_j)
                    for j in range(NJ):
                        for c in range(NC_E):
                            ps = tpps.tile([128, 128], dt.bfloat16, tag="tp")
                            nc.tensor.transpose(
                                ps[:], xtoks[j][:, c * 128:(c + 1) * 128],
                                idb)
                            nc.vector.tensor_copy(
                                out=xt_s[c][:, t0 + j * 128:
                                            t0 + (j + 1) * 128],
                                in_=ps[:])
                    # -- qT = elu(Wq^T xT + bq)+1 --
                    for co in range(NC_E):
                        qps = paps.tile([128, TT], dt.float32, tag="mm")
                        for ci in range(NC_E):
                            nc.tensor.matmul(
                                qps[:],
                                wq_s[:, ci * E + co * 128:
                                     ci * E + (co + 1) * 128],
                                xt_s[ci][:, t0:t0 + TT],
                                start=(ci == 0), stop=(ci == NC_E - 1))
                        t1 = pat.tile([128, TT], dt.bfloat16, tag="t1")
                        t2 = pat.tile([128, TT], dt.bfloat16, tag="t2")
                        nc.scalar.activation(t1[:], qps[:], AF.Relu,
                                             bias=bq_c(co))
                        nc.vector.tensor_scalar(
                            out=t2[:], in0=qps[:], scalar1=bq_c(co),
                            scalar2=0.0, op0=ALU.add, op1=ALU.min)
                        nc.scalar.activation(t2[:], t2[:], AF.Exp)
                        nc.vector.tensor_add(
                            qt_s[co][:, t0:t0 + TT], t1[:], t2[:])
                    # -- K, V token-major; accumulate KVT, Ksum --
                    for j in range(NJ):
                        kps = paps.tile([128, E], dt.float32, tag="mm")
                        nc.tensor.matmul(kps[:], ones1x128, bk_s[:],
                                         start=True, stop=False,
                                         skip_group_check=True)
                        for ci in range(NC_E):
                            nc.tensor.matmul(
                                kps[:],
                                xt_s[ci][:, t0 + j * 128: t0 + (j + 1) * 128],
                                wk_s[:, ci * E:(ci + 1) * E],
                                start=False, stop=(ci == NC_E - 1),
                                skip_group_check=True)
                        kt = pakv.tile([128, E], dt.bfloat16, tag="kt")
                        t1 = pat.tile([128, E], dt.bfloat16, tag="t1")
                        nc.scalar.activation(t1[:], kps[:], AF.Relu)
                        nc.vector.tensor_scalar_min(kt[:], kps[:], 0.0)
                        nc.scalar.activation(kt[:], kt[:], AF.Exp)
                        nc.vector.tensor_add(kt[:], kt[:], t1[:])

                        vps = paps.tile([128, E], dt.float32, tag="mm")
                        nc.tensor.matmul(vps[:], ones1x128, bv_s[:],
                                         start=True, stop=False,
                                         skip_group_check=True)
                        for ci in range(NC_E):
                            nc.tensor.matmul(
                                vps[:],
                                xt_s[ci][:, t0 + j * 128: t0 + (j + 1) * 128],
                                wv_s[:, ci * E:(ci + 1) * E],
                                start=False, stop=(ci == NC_E - 1),
                                skip_group_check=True)
                        vt = pakv.tile([128, E], dt.bfloat16, tag="vt")
                        nc.scalar.activation(vt[:], vps[:], AF.Copy)

                        last_kv = (t == NT - 1) and (j == NJ - 1)
                        for c in range(NC_E):
                            nc.tensor.matmul(
                                kvt_ps[:, c * 128:(c + 1) * 128],
                                kt[:, c * 128:(c + 1) * 128],
                                vt[:, c * 128:(c + 1) * 128],
                                start=first_kv, stop=last_kv,
                                skip_group_check=True)
                        nc.tensor.matmul(ksum_ps[:], onesc, kt[:],
                                         start=first_kv, stop=last_kv,
                                         skip_group_check=True)
                        first_kv = False

                # ---- extract blockdiag KVT and Ksum^T chunks ----
                nc.vector.memset(kvt_s[:], 0.0)
                for c in range(NC_E):
                    for h in range(2):
                        o = c * 128 + h * 64
                        nc.vector.tensor_copy(
                            out=kvt_s[h * 64:(h + 1) * 64, o:o + 64],
                            in_=kvt_ps[h * 64:(h + 1) * 64, o:o + 64])
                nc.scalar.activation(ksumb_s[:], ksum_ps[:], AF.Copy)
                for c in range(NC_E):
                    ps = tpps.tile([128, 1], dt.float32, tag="tpks")
                    nc.tensor.matmul(ps[0:128, 0:1],
                                     ksumb_s[0:1, c * 128:(c + 1) * 128],
                                     onesr_s[0:1, 0:1],
                                     start=True, stop=True)
                    nc.vector.tensor_copy(out=ksc_s[:, c:c + 1],
                                          in_=ps[0:128, 0:1])

            # =========================== PHASE B ==========================
            with tc.tile_pool(name="pb_ps", bufs=2, space="PSUM") as pbps, \
                 tc.tile_pool(name="pb_bc", bufs=2, space="PSUM") as pbbc, \
                 tc.tile_pool(name="pb_st", bufs=2, space="PSUM") as pbst, \
                 tc.tile_pool(name="tp2_ps", bufs=1, space="PSUM") as tpps2, \
                 tc.tile_pool(name="pb_sb", bufs=2, space="SBUF") as pbsb, \
                 tc.tile_pool(name="pb_q", bufs=4, space="SBUF") as pbq, \
                 tc.tile_pool(name="pb_x1", bufs=4, space="SBUF") as pbx1, \
                 tc.tile_pool(name="pb_h", bufs=NC_H, space="SBUF") as pbh, \
                 tc.tile_pool(name="pb_o", bufs=4, space="SBUF") as pbo:

                for t in range(NT):
                    t0 = t * TT
                    # ---- Z and QZ ----
                    qzts = []
                    for c in range(NC_E):
                        qks = pbsb.tile([128, TT], dt.bfloat16, tag="qks")
                        nc.vector.tensor_scalar_mul(
                            qks[:], qt_s[c][:, t0:t0 + TT], ksc_s[:, c:c + 1])
                        zden = pbst.tile([2, TT], dt.float32, tag="st2", bufs=1)
                        nc.tensor.matmul(zden[:], hsel, qks[:],
                                         start=True, stop=True)
                        zt = pbsb.tile([2, TT], dt.float32, tag="zt")
                        nc.vector.tensor_scalar_add(zt[:], zden[:], ATTN_EPS)
                        nc.vector.reciprocal(zt[:], zt[:])
                        ztb = pbsb.tile([2, TT], dt.bfloat16, tag="ztb")
                        nc.scalar.activation(ztb[:], zt[:], AF.Copy)
                        zb = pbbc.tile([128, TT], dt.float32, tag="bc")
                        nc.tensor.matmul(zb[:], tsb[0:2, :], ztb[:],
                                         start=True, stop=True)
                        qzt = pbq.tile([128, TT], dt.bfloat16, tag="qzt")
                        nc.vector.tensor_mul(qzt[:], qt_s[c][:, t0:t0 + TT],
                                             zb[:])
                        qzts.append(qzt)
                    # ---- attention ----
                    att_sb = []
                    for c in range(NC_E):
                        aps = pbps.tile([128, TT], dt.float32, tag="mm")
                        nc.tensor.matmul(aps[:],
                                         kvt_s[:, c * 128:(c + 1) * 128],
                                         qzts[c][:], start=True, stop=True)
                        asb = pbq.tile([128, TT], dt.bfloat16, tag="asb")
                        nc.scalar.activation(asb[:], aps[:], AF.Copy)
                        att_sb.append(asb)
                    # ---- Wo + residual + LN1 stats ----
                    h1ts = []
                    ssum1 = pbst.tile([1, TT], dt.float32, tag="st1")
                    ssq1 = pbst.tile([1, TT], dt.float32, tag="st1")
                    for co in range(NC_E):
                        ops_ = pbps.tile([128, TT], dt.float32, tag="mm")
                        for ci in range(NC_E):
                            nc.tensor.matmul(
                                ops_[:],
                                wo_s[:, ci * E + co * 128:
                                     ci * E + (co + 1) * 128],
                                att_sb[ci][:],
                                start=(ci == 0), stop=(ci == NC_E - 1))
                        h1t = pbx1.tile([128, TT], dt.bfloat16, tag="h1")
                        nc.vector.scalar_tensor_tensor(
                            out=h1t[:], in0=ops_[:], scalar=bo_c(co),
                            in1=xt_s[co][:, t0:t0 + TT],
                            op0=ALU.add, op1=ALU.add)
                        h1ts.append(h1t)
                        sq = pbsb.tile([128, TT], dt.bfloat16, tag="sq")
                        nc.vector.tensor_mul(sq[:], h1t[:], h1t[:])
                        nc.tensor.matmul(ssum1[:], onesc, h1t[:],
                                         start=(co == 0),
                                         stop=(co == NC_E - 1),
                                         skip_group_check=True)
                        nc.tensor.matmul(ssq1[:], onesc, sq[:],
                                         start=(co == 0),
                                         stop=(co == NC_E - 1),
                                         skip_group_check=True)
                    x1ts = _ln_norm(nc, pbsb, pbbc, pbx1, h1ts, ssum1, ssq1,
                                    onesr_s, g1_c, be1_c, "x1")
                    # ---- FFN ----
                    hts = []
                    for j in range(NC_H):
                        hps = pbps.tile([128, TT], dt.float32, tag="mm")
                        for ci in range(NC_E):
                            nc.tensor.matmul(
                                hps[:],
                                w1_s[:, ci * HID + j * 128:
                                     ci * HID + (j + 1) * 128],
                                x1ts[ci][:],
                                start=(ci == 0), stop=(ci == NC_E - 1))
                        ht = pbh.tile([128, TT], dt.bfloat16, tag="ht")
                        nc.scalar.activation(ht[:], hps[:], AF.Relu,
                                             bias=b1_c(j))
                        hts.append(ht)
                    h2ts = []
                    ssum2 = pbst.tile([1, TT], dt.float32, tag="st1")
                    ssq2 = pbst.tile([1, TT], dt.float32, tag="st1")
                    for co in range(NC_E):
                        ops2 = pbps.tile([128, TT], dt.float32, tag="mm")
                        for j in range(NC_H):
                            nc.tensor.matmul(
                                ops2[:],
                                w2_s[:, j * E + co * 128:
                                     j * E + (co + 1) * 128],
                                hts[j][:],
                                start=(j == 0), stop=(j == NC_H - 1))
                        h2t = pbo.tile([128, TT], dt.bfloat16, tag="h2")
                        nc.vector.scalar_tensor_tensor(
                            out=h2t[:], in0=ops2[:], scalar=b2_c(co),
                            in1=x1ts[co][:], op0=ALU.add, op1=ALU.add)
                        h2ts.append(h2t)
                        sq = pbsb.tile([128, TT], dt.bfloat16, tag="sq")
                        nc.vector.tensor_mul(sq[:], h2t[:], h2t[:])
                        nc.tensor.matmul(ssum2[:], onesc, h2t[:],
                                         start=(co == 0),
                                         stop=(co == NC_E - 1),
                                         skip_group_check=True)
                        nc.tensor.matmul(ssq2[:], onesc, sq[:],
                                         start=(co == 0),
                                         stop=(co == NC_E - 1),
                                         skip_group_check=True)
                    outs = _ln_norm(nc, pbsb, pbbc, pbo, h2ts, ssum2, ssq2,
                                    onesr_s, g2_c, be2_c, "ou")
                    # ---- transpose back to token-major, DMA out ----
                    for j in range(NJ):
                        otok = pbsb.tile([128, E], dt.int8, tag="otok")
                        for c in range(NC_E):
                            ps = tpps2.tile([128, 128], dt.bfloat16, tag="tp2")
                            nc.tensor.transpose(
                                ps[:], outs[c][:, j * 128:(j + 1) * 128],
                                idb)
                            nc.vector.tensor_scalar_mul(
                                otok[:, c * 128:(c + 1) * 128], ps[:], OSCALE)
                        nc.sync.dma_start(
                            out=out_d[t0 + j * 128: t0 + (j + 1) * 128, :],
                            in_=otok[:])

    nc.compile()
    return nc


def _prep_in_maps(inputs):
    x = np.asarray(inputs["x"], dtype=F32)
    blob8 = _CACHE.get("blob8")
    if blob8 is None:
        blob8 = _CACHE["blob8"] = np.zeros((NCORES, BLOB_ROWS, 512),
                                           np.uint8)
        _CACHE["xq_tmp"] = np.empty((B, S, E), np.float32)
    tmp = _CACHE["xq_tmp"]

    # x -> int8 rows 0:4096 (tmp holds exact integers, copyto truncates)
    np.multiply(x, XSCALE, out=tmp)
    np.rint(tmp, out=tmp)
    np.clip(tmp, -127, 127, out=tmp)
    np.copyto(blob8[:, :S, :].view(np.int8), tmp, casting="unsafe")

    # weight shards
    wa = np.concatenate([
        np.asarray(inputs["Wq"], F32).astype(BF16),
        np.asarray(inputs["Wk"], F32).astype(BF16),
        np.asarray(inputs["Wv"], F32).astype(BF16),
        np.asarray(inputs["Wo"], F32).astype(BF16),
        np.asarray(inputs["W2"], F32).astype(BF16),
    ], axis=0)                                     # [4*E+HID, E]
    wbb = np.asarray(inputs["W1"], F32).astype(BF16)   # [E, HID]
    for b in range(NCORES):
        blob8[b, WAOFF:WBOFF] = \
            wa[b * 512:(b + 1) * 512].view(np.uint8).reshape(1024, 512)
        blob8[b, WBOFF:AUXOFF] = \
            wbb[b * 64:(b + 1) * 64].view(np.uint8).reshape(512, 512)

    # aux page [128, 256] bf16
    aux = np.zeros((128, 256), dtype=BF16)
    aux[:, 0:128] = np.eye(128, dtype=np.float32).astype(BF16)
    aux[0:64, 128] = BF16(1.0)
    aux[64:128, 129] = BF16(1.0)
    aux[:, 130] = BF16(1.0)
    aux[:, 131:135] = np.asarray(inputs["bk"], F32).astype(BF16) \
        .reshape(4, 128).T
    aux[:, 135:139] = np.asarray(inputs["bv"], F32).astype(BF16) \
        .reshape(4, 128).T
    pp = np.zeros((128, 44), dtype=F32)
    for c in range(4):
        pp[:, c] = inputs["bq"][c * 128:(c + 1) * 128]
        pp[:, 4 + c] = inputs["bo"][c * 128:(c + 1) * 128]
        pp[:, 24 + c] = inputs["b2"][c * 128:(c + 1) * 128]
        pp[:, 28 + c] = inputs["g1"][c * 128:(c + 1) * 128]
        pp[:, 32 + c] = inputs["be1"][c * 128:(c + 1) * 128]
        pp[:, 36 + c] = inputs["g2"][c * 128:(c + 1) * 128]
        pp[:, 40 + c] = inputs["be2"][c * 128:(c + 1) * 128]
    for j in range(16):
        pp[:, 8 + j] = inputs["b1"][j * 128:(j + 1) * 128]
    hi = pp.astype(BF16)
    aux[:, 139:183] = hi
    aux[:, 183:227] = (pp - hi.astype(F32)).astype(BF16)
    blob8[:, AUXOFF:] = aux.view(np.uint8)[None]

    return [{"blob": blob8[b]} for b in range(NCORES)]


def kernel(**inputs):
    if "nc" not in _CACHE:
        import jax
        try:
            import tempfile
            jax.config.update("jax_compilation_cache_dir",
                              tempfile.gettempdir() + "/jaxcache_bass")
            jax.config.update("jax_persistent_cache_min_compile_time_secs",
                              0.0)
            jax.config.update("jax_persistent_cache_min_entry_size_bytes", 0)
        except Exception:
            pass
        _CACHE["nc"] = _build()
    nc = _CACHE["nc"]
    in_maps = _prep_in_maps(inputs)

    res = run_bass_kernel_spmd(nc, in_maps, core_ids=list(range(NCORES)),
                               **_CACHE.get("run_kwargs", {}))
    _CACHE["last"] = res
    outs = np.stack([np.asarray(res.results[b]["out"])
                     for b in range(NCORES)], axis=0)
    return np.multiply(outs, F32(1.0 / OSCALE), dtype=F32)



# revision 36
# speedup vs baseline: 1.0361x; 1.0361x over previous
"""Trainium2 Bass kernel for a linear-attention transformer block.

B=8, S=4096, E=512, NH=8, DH=64, HID=2048.
Sharding: data-parallel over batch — one batch element per NeuronCore.
The axon tunnel (~40MB/s, half-duplex, large per-operand overhead)
dominates wall time, so bytes AND operand count are minimized — each
core receives ONE uint8 blob:
  - x int8-quantized on host (scale 32, RNE; x~N(0,1) so the added
    output error is ~9.5e-3), dequantized to bf16 on device;
  - weight shards (1/8 of [wq;wk;wv;wo;w2] and of w1, bf16 bytes),
    AllGathered on-device (HBM-to-HBM collectives via DRAM bounce
    buffers), loaded to SBUF through bitcast DMAs;
  - an aux page holding identity/selectors plus bk/bv as columns and
    the f32 param table split into bf16 hi+lo halves; the ones row,
    head-expand matrix, bk/bv rows and f32 params are reconstructed
    on device (memset / PE transpose / vector add).
The output is int8 (scale 28; post-LN rows are ~unit-variance), host
upcasts to f32; the donated zero output buffer shrinks equally.
Total rel err ~1.49e-2 (gate 2e-2), deterministic. A persistent JAX
compilation cache removes the per-call XLA recompile that
run_bass_kernel_spmd's fresh-closure jit otherwise incurs. Steady-state
matches an I/O-identical no-compute kernel: transfer-floor bound.

Per-core pipeline (feature-major activations, bf16 matmuls, f32 PSUM):
  phase A: x -> xT (PE transpose); qT = elu(Wq^T xT + bq)+1 stored; K,V
           token-major; KVT[d,m] and Ksum accumulated in PSUM over all S.
  phase B: Z = 1/(Q.Ksum+eps); attnT = blockdiag(KVT) @ (Q*Z); Wo; LN1
           (stats via ones-matmuls); FFN; LN2; PE-transpose out.
"""

import numpy as np
import ml_dtypes

from concourse import bass, bacc, tile, mybir
from concourse.bass_utils import run_bass_kernel_spmd

BF16 = ml_dtypes.bfloat16
F32 = np.float32

B, S, E, NH, HID, DH = 8, 4096, 512, 8, 2048, 64
ATTN_EPS = 1e-6
LN_EPS = 1e-5

NCORES = 8
OSCALE = 28.0             # int8 output quantization scale
XSCALE = 32.0             # int8 input quantization scale
WAOFF, WBOFF, AUXOFF = 4096, 5120, 5632   # blob row offsets
BLOB_ROWS = 5760
TT = 512                  # tokens per tile
NT = S // TT              # 8 token tiles
NC_E = E // 128           # 4 feature chunks
NC_H = HID // 128         # 16 hidden chunks
NJ = TT // 128            # 4 token sub-tiles per tile

dt = mybir.dt
AF = mybir.ActivationFunctionType
ALU = mybir.AluOpType

_CACHE = {}


def _ln_norm(nc, pbsb, pbbc, opool, hts, ssum, ssq, onesr_s, g_c, be_c, otag):
    """LayerNorm: per-chunk feature-major tiles + sum/sumsq stats psums."""
    inv = 1.0 / E
    mean = pbsb.tile([1, TT], dt.float32, tag="mean")
    nc.vector.tensor_scalar_mul(mean[:], ssum[:], inv)
    msq = pbsb.tile([1, TT], dt.float32, tag="msq")
    nc.vector.tensor_mul(msq[:], mean[:], mean[:])
    var = pbsb.tile([1, TT], dt.float32, tag="var")
    nc.vector.tensor_scalar(out=var[:], in0=ssq[:], scalar1=inv,
                            scalar2=LN_EPS, op0=ALU.mult, op1=ALU.add)
    nc.vector.tensor_sub(var[:], var[:], msq[:])
    rs = pbsb.tile([1, TT], dt.float32, tag="rs")
    nc.vector.reciprocal(rs[:], var[:])
    nc.scalar.activation(rs[:], rs[:], AF.Sqrt)
    mean_b = pbsb.tile([1, TT], dt.bfloat16, tag="meanb")
    nc.scalar.activation(mean_b[:], mean[:], AF.Copy)
    rs_b = pbsb.tile([1, TT], dt.bfloat16, tag="rsb")
    nc.scalar.activation(rs_b[:], rs[:], AF.Copy)
    mb = pbbc.tile([128, TT], dt.float32, tag="bc")
    nc.tensor.matmul(mb[:], onesr_s[0:1, 0:128], mean_b[:],
                     start=True, stop=True)
    rb = pbbc.tile([128, TT], dt.float32, tag="bc")
    nc.tensor.matmul(rb[:], onesr_s[0:1, 0:128], rs_b[:],
                     start=True, stop=True)
    outs = []
    for c in range(len(hts)):
        tmp = pbsb.tile([128, TT], dt.bfloat16, tag="nrm")
        nc.vector.tensor_sub(tmp[:], hts[c][:], mb[:])
        nc.vector.tensor_mul(tmp[:], tmp[:], rb[:])
        o = opool.tile([128, TT], dt.bfloat16, tag=otag)
        nc.scalar.activation(o[:], tmp[:], AF.Identity,
                             bias=be_c(c), scale=g_c(c))
        outs.append(o)
    return outs


def _build():
    nc = bacc.Bacc("TRN2", target_bir_lowering=False, debug=False,
                   num_devices=NCORES)

    def din(name, shape, d):
        return nc.dram_tensor(name, list(shape), d, kind="ExternalInput")

    # Single input operand per core (per-operand tunnel overhead is large):
    # uint8 blob, 512-byte rows.
    #   rows 0:4096      x int8 [4096, 512]
    #   rows 4096:5120   wa shard: 1/8 of [wq;wk;wv;wo;w2] ([512,512] bf16)
    #   rows 5120:5632   wb shard: 1/8 of w1 ([64,2048] bf16)
    #   rows 5632:5760   aux page [128, 256] bf16:
    #     cols 0:128 identity, 128:130 headsel, 130 ones, 131:135 bk,
    #     135:139 bv, 139:183 pp-hi, 183:227 pp-lo
    # pp cols: 0-3 bq, 4-7 bo, 8-23 b1, 24-27 b2, 28-31 g1, 32-35 be1,
    #          36-39 g2, 40-43 be2 (f32 = hi + lo reconstructed on device)
    blob_d = din("blob", (BLOB_ROWS, 512), dt.uint8)
    out_d = nc.dram_tensor("out", [S, E], dt.int8, kind="ExternalOutput")

    with tile.TileContext(nc) as tc:
        from contextlib import ExitStack
        es = ExitStack()
        with es:
            dpool = es.enter_context(
                tc.tile_pool(name="wdram", bufs=1, space="DRAM"))
            cpool = es.enter_context(tc.tile_pool(name="const", bufs=1))

            # ---- AllGather the weight shards into full DRAM copies ----
            # boutA rows are weight rows (1024B each, bitcast to bf16 on
            # the SBUF loads); boutB rows are w1 rows (4096B each).
            rg = [list(range(NCORES))]
            binA = dpool.tile([1024, 512], dt.uint8, tag="biA", name="biA")
            boutA = dpool.tile([4 * E + HID, 1024], dt.uint8,
                               tag="boA", name="boA")
            nc.gpsimd.dma_start(binA[:], blob_d[WAOFF:WBOFF, :])
            nc.gpsimd.collective_compute(
                "AllGather", mybir.AluOpType.bypass, replica_groups=rg,
                ins=[binA.opt()], outs=[boutA.opt()])
            binB = dpool.tile([512, 512], dt.uint8, tag="biB", name="biB")
            boutB = dpool.tile([E, 4096], dt.uint8, tag="boB", name="boB")
            nc.gpsimd.dma_start(binB[:], blob_d[WBOFF:AUXOFF, :])
            nc.gpsimd.collective_compute(
                "AllGather", mybir.AluOpType.bypass, replica_groups=rg,
                ins=[binB.opt()], outs=[boutB.opt()])
            OQ, OK_, OV, OO, O2 = 0, E, 2 * E, 3 * E, 4 * E

            wq_s = cpool.tile([128, NC_E * E], dt.bfloat16, tag="wq")
            wk_s = cpool.tile([128, NC_E * E], dt.bfloat16, tag="wk")
            wv_s = cpool.tile([128, NC_E * E], dt.bfloat16, tag="wv")
            wo_s = cpool.tile([128, NC_E * E], dt.bfloat16, tag="wo")
            w1_s = cpool.tile([128, NC_E * HID], dt.bfloat16, tag="w1")
            w2_s = cpool.tile([128, NC_H * E], dt.bfloat16, tag="w2")
            pp_s = cpool.tile([128, 44], dt.float32, tag="pp")
            aux_s = cpool.tile([128, 256], dt.bfloat16, tag="aux")
            tsb = cpool.tile([11, 128], dt.bfloat16, tag="tsb")
            onesr_s = cpool.tile([1, TT], dt.bfloat16, tag="onesr")
            bk_s = cpool.tile([1, E], dt.bfloat16, tag="bk")
            bv_s = cpool.tile([1, E], dt.bfloat16, tag="bv")
            qt_s = [cpool.tile([128, S], dt.bfloat16, tag=f"qt{c}", name=f"qt{c}")
                    for c in range(NC_E)]
            xt_s = [cpool.tile([128, S], dt.bfloat16, tag=f"xt{c}", name=f"xt{c}")
                    for c in range(NC_E)]
            kvt_s = cpool.tile([128, NC_E * 128], dt.bfloat16, tag="kvt")
            ksumb_s = cpool.tile([1, E], dt.bfloat16, tag="ksumb")
            ksc_s = cpool.tile([128, NC_E], dt.float32, tag="ksc")

            for c in range(NC_E):
                nc.sync.dma_start(
                    out=wq_s[:, c * E:(c + 1) * E],
                    in_=boutA[OQ + c * 128:OQ + (c + 1) * 128, :]
                    .bitcast(dt.bfloat16))
                nc.sync.dma_start(
                    out=wk_s[:, c * E:(c + 1) * E],
                    in_=boutA[OK_ + c * 128:OK_ + (c + 1) * 128, :]
                    .bitcast(dt.bfloat16))
                nc.sync.dma_start(
                    out=wv_s[:, c * E:(c + 1) * E],
                    in_=boutA[OV + c * 128:OV + (c + 1) * 128, :]
                    .bitcast(dt.bfloat16))
                nc.sync.dma_start(
                    out=wo_s[:, c * E:(c + 1) * E],
                    in_=boutA[OO + c * 128:OO + (c + 1) * 128, :]
                    .bitcast(dt.bfloat16))
                nc.sync.dma_start(
                    out=w1_s[:, c * HID:(c + 1) * HID],
                    in_=boutB[c * 128:(c + 1) * 128, :].bitcast(dt.bfloat16))
            for j in range(NC_H):
                nc.sync.dma_start(
                    out=w2_s[:, j * E:(j + 1) * E],
                    in_=boutA[O2 + j * 128:O2 + (j + 1) * 128, :]
                    .bitcast(dt.bfloat16))
            nc.sync.dma_start(out=aux_s[:],
                              in_=blob_d[AUXOFF:AUXOFF + 128, :]
                              .bitcast(dt.bfloat16))

            idb = aux_s[:, 0:128]            # bf16 identity
            hsel = aux_s[:, 128:130]         # [128,2] head select
            onesc = aux_s[:, 130:131]        # [128,1] ones col
            ones1x128 = onesr_s[0:1, 0:128]  # [1,128]

            # reconstruct small params on device (saves tunnel operands):
            # pp f32 = hi + lo halves; ones row via memset; hexp/bk/bv via
            # PE transpose of aux cols 128:139 -> [11,128].
            nc.vector.tensor_add(pp_s[:], aux_s[:, 139:183],
                                 aux_s[:, 183:227])
            nc.vector.memset(onesr_s[:], 1.0)
            with tc.tile_pool(name="init_ps", bufs=1, space="PSUM") as ips:
                tp0 = ips.tile([11, 128], dt.bfloat16, tag="tp0")
                nc.tensor.transpose(tp0[:], aux_s[:, 128:139], idb)
                nc.scalar.activation(tsb[:], tp0[:], AF.Copy)
            for c in range(NC_E):
                nc.sync.dma_start(out=bk_s[0:1, c * 128:(c + 1) * 128],
                                  in_=tsb[3 + c:4 + c, :])
                nc.sync.dma_start(out=bv_s[0:1, c * 128:(c + 1) * 128],
                                  in_=tsb[7 + c:8 + c, :])

            bq_c = lambda c: pp_s[:, c:c + 1]
            bo_c = lambda c: pp_s[:, 4 + c:5 + c]
            b1_c = lambda j: pp_s[:, 8 + j:9 + j]
            b2_c = lambda c: pp_s[:, 24 + c:25 + c]
            g1_c = lambda c: pp_s[:, 28 + c:29 + c]
            be1_c = lambda c: pp_s[:, 32 + c:33 + c]
            g2_c = lambda c: pp_s[:, 36 + c:37 + c]
            be2_c = lambda c: pp_s[:, 40 + c:41 + c]

            # =========================== PHASE A ==========================
            with tc.tile_pool(name="acc_ps", bufs=1, space="PSUM") as accp, \
                 tc.tile_pool(name="pa_ps", bufs=2, space="PSUM") as paps, \
                 tc.tile_pool(name="tp_ps", bufs=2, space="PSUM") as tpps, \
                 tc.tile_pool(name="pa_x", bufs=4, space="SBUF") as pax, \
                 tc.tile_pool(name="pa_t", bufs=2, space="SBUF") as pat, \
                 tc.tile_pool(name="pa_kv", bufs=3, space="SBUF") as pakv:

                kvt_ps = accp.tile([128, NC_E * 128], dt.float32, tag="kvtp")
                ksum_ps = accp.tile([1, E], dt.float32, tag="ksump")

                first_kv = True
                for t in range(NT):
                    t0 = t * TT
                    xtoks = []
                    for j in range(NJ):
                        xq_j = pax.tile([128, E], dt.int8, tag="xtokq")
                        nc.sync.dma_start(
                            out=xq_j[:],
                            in_=blob_d[t0 + j * 128: t0 + (j + 1) * 128, :]
                            .bitcast(dt.int8))
                        xt_j = pax.tile([128, E], dt.bfloat16, tag="xtok")
                        nc.vector.tensor_scalar_mul(xt_j[:], xq_j[:],
                                                    1.0 / XSCALE)
                        xtoks.append(xt_j)
                    for j in range(NJ):
                        for c in range(NC_E):
                            ps = tpps.tile([128, 128], dt.bfloat16, tag="tp")
                            nc.tensor.transpose(
                                ps[:], xtoks[j][:, c * 128:(c + 1) * 128],
                                idb)
                            nc.vector.tensor_copy(
                                out=xt_s[c][:, t0 + j * 128:
                                            t0 + (j + 1) * 128],
                                in_=ps[:])
                    # -- qT = elu(Wq^T xT + bq)+1 --
                    for co in range(NC_E):
                        qps = paps.tile([128, TT], dt.float32, tag="mm")
                        for ci in range(NC_E):
                            nc.tensor.matmul(
                                qps[:],
                                wq_s[:, ci * E + co * 128:
                                     ci * E + (co + 1) * 128],
                                xt_s[ci][:, t0:t0 + TT],
                                start=(ci == 0), stop=(ci == NC_E - 1))
                        t1 = pat.tile([128, TT], dt.bfloat16, tag="t1")
                        t2 = pat.tile([128, TT], dt.bfloat16, tag="t2")
                        nc.scalar.activation(t1[:], qps[:], AF.Relu,
                                             bias=bq_c(co))
                        nc.vector.tensor_scalar(
                            out=t2[:], in0=qps[:], scalar1=bq_c(co),
                            scalar2=0.0, op0=ALU.add, op1=ALU.min)
                        nc.scalar.activation(t2[:], t2[:], AF.Exp)
                        nc.vector.tensor_add(
                            qt_s[co][:, t0:t0 + TT], t1[:], t2[:])
                    # -- K, V token-major; accumulate KVT, Ksum --
                    for j in range(NJ):
                        kps = paps.tile([128, E], dt.float32, tag="mm")
                        nc.tensor.matmul(kps[:], ones1x128, bk_s[:],
                                         start=True, stop=False,
                                         skip_group_check=True)
                        for ci in range(NC_E):
                            nc.tensor.matmul(
                                kps[:],
                                xt_s[ci][:, t0 + j * 128: t0 + (j + 1) * 128],
                                wk_s[:, ci * E:(ci + 1) * E],
                                start=False, stop=(ci == NC_E - 1),
                                skip_group_check=True)
                        kt = pakv.tile([128, E], dt.bfloat16, tag="kt")
                        t1 = pat.tile([128, E], dt.bfloat16, tag="t1")
                        nc.scalar.activation(t1[:], kps[:], AF.Relu)
                        nc.vector.tensor_scalar_min(kt[:], kps[:], 0.0)
                        nc.scalar.activation(kt[:], kt[:], AF.Exp)
                        nc.vector.tensor_add(kt[:], kt[:], t1[:])

                        vps = paps.tile([128, E], dt.float32, tag="mm")
                        nc.tensor.matmul(vps[:], ones1x128, bv_s[:],
                                         start=True, stop=False,
                                         skip_group_check=True)
                        for ci in range(NC_E):
                            nc.tensor.matmul(
                                vps[:],
                                xt_s[ci][:, t0 + j * 128: t0 + (j + 1) * 128],
                                wv_s[:, ci * E:(ci + 1) * E],
                                start=False, stop=(ci == NC_E - 1),
                                skip_group_check=True)
                        vt = pakv.tile([128, E], dt.bfloat16, tag="vt")
                        nc.scalar.activation(vt[:], vps[:], AF.Copy)

                        last_kv = (t == NT - 1) and (j == NJ - 1)
                        for c in range(NC_E):
                            nc.tensor.matmul(
                                kvt_ps[:, c * 128:(c + 1) * 128],
                                kt[:, c * 128:(c + 1) * 128],
                                vt[:, c * 128:(c + 1) * 128],
                                start=first_kv, stop=last_kv,
                                skip_group_check=True)
                        nc.tensor.matmul(ksum_ps[:], onesc, kt[:],
                                         start=first_kv, stop=last_kv,
                                         skip_group_check=True)
                        first_kv = False

                # ---- extract blockdiag KVT and Ksum^T chunks ----
                nc.vector.memset(kvt_s[:], 0.0)
                for c in range(NC_E):
                    for h in range(2):
                        o = c * 128 + h * 64
                        nc.vector.tensor_copy(
                            out=kvt_s[h * 64:(h + 1) * 64, o:o + 64],
                            in_=kvt_ps[h * 64:(h + 1) * 64, o:o + 64])
                nc.scalar.activation(ksumb_s[:], ksum_ps[:], AF.Copy)
                for c in range(NC_E):
                    ps = tpps.tile([128, 1], dt.float32, tag="tpks")
                    nc.tensor.matmul(ps[0:128, 0:1],
                                     ksumb_s[0:1, c * 128:(c + 1) * 128],
                                     onesr_s[0:1, 0:1],
                                     start=True, stop=True)
                    nc.vector.tensor_copy(out=ksc_s[:, c:c + 1],
                                          in_=ps[0:128, 0:1])

            # =========================== PHASE B ==========================
            with tc.tile_pool(name="pb_ps", bufs=2, space="PSUM") as pbps, \
                 tc.tile_pool(name="pb_bc", bufs=2, space="PSUM") as pbbc, \
                 tc.tile_pool(name="pb_st", bufs=2, space="PSUM") as pbst, \
                 tc.tile_pool(name="tp2_ps", bufs=1, space="PSUM") as tpps2, \
                 tc.tile_pool(name="pb_sb", bufs=2, space="SBUF") as pbsb, \
                 tc.tile_pool(name="pb_q", bufs=4, space="SBUF") as pbq, \
                 tc.tile_pool(name="pb_x1", bufs=4, space="SBUF") as pbx1, \
                 tc.tile_pool(name="pb_h", bufs=NC_H, space="SBUF") as pbh, \
                 tc.tile_pool(name="pb_o", bufs=4, space="SBUF") as pbo:

                for t in range(NT):
                    t0 = t * TT
                    # ---- Z and QZ ----
                    qzts = []
                    for c in range(NC_E):
                        qks = pbsb.tile([128, TT], dt.bfloat16, tag="qks")
                        nc.vector.tensor_scalar_mul(
                            qks[:], qt_s[c][:, t0:t0 + TT], ksc_s[:, c:c + 1])
                        zden = pbst.tile([2, TT], dt.float32, tag="st2", bufs=1)
                        nc.tensor.matmul(zden[:], hsel, qks[:],
                                         start=True, stop=True)
                        zt = pbsb.tile([2, TT], dt.float32, tag="zt")
                        nc.vector.tensor_scalar_add(zt[:], zden[:], ATTN_EPS)
                        nc.vector.reciprocal(zt[:], zt[:])
                        ztb = pbsb.tile([2, TT], dt.bfloat16, tag="ztb")
                        nc.scalar.activation(ztb[:], zt[:], AF.Copy)
                        zb = pbbc.tile([128, TT], dt.float32, tag="bc")
                        nc.tensor.matmul(zb[:], tsb[0:2, :], ztb[:],
                                         start=True, stop=True)
                        qzt = pbq.tile([128, TT], dt.bfloat16, tag="qzt")
                        nc.vector.tensor_mul(qzt[:], qt_s[c][:, t0:t0 + TT],
                                             zb[:])
                        qzts.append(qzt)
                    # ---- attention ----
                    att_sb = []
                    for c in range(NC_E):
                        aps = pbps.tile([128, TT], dt.float32, tag="mm")
                        nc.tensor.matmul(aps[:],
                                         kvt_s[:, c * 128:(c + 1) * 128],
                                         qzts[c][:], start=True, stop=True)
                        asb = pbq.tile([128, TT], dt.bfloat16, tag="asb")
                        nc.scalar.activation(asb[:], aps[:], AF.Copy)
                        att_sb.append(asb)
                    # ---- Wo + residual + LN1 stats ----
                    h1ts = []
                    ssum1 = pbst.tile([1, TT], dt.float32, tag="st1")
                    ssq1 = pbst.tile([1, TT], dt.float32, tag="st1")
                    for co in range(NC_E):
                        ops_ = pbps.tile([128, TT], dt.float32, tag="mm")
                        for ci in range(NC_E):
                            nc.tensor.matmul(
                                ops_[:],
                                wo_s[:, ci * E + co * 128:
                                     ci * E + (co + 1) * 128],
                                att_sb[ci][:],
                                start=(ci == 0), stop=(ci == NC_E - 1))
                        h1t = pbx1.tile([128, TT], dt.bfloat16, tag="h1")
                        nc.vector.scalar_tensor_tensor(
                            out=h1t[:], in0=ops_[:], scalar=bo_c(co),
                            in1=xt_s[co][:, t0:t0 + TT],
                            op0=ALU.add, op1=ALU.add)
                        h1ts.append(h1t)
                        sq = pbsb.tile([128, TT], dt.bfloat16, tag="sq")
                        nc.vector.tensor_mul(sq[:], h1t[:], h1t[:])
                        nc.tensor.matmul(ssum1[:], onesc, h1t[:],
                                         start=(co == 0),
                                         stop=(co == NC_E - 1),
                                         skip_group_check=True)
                        nc.tensor.matmul(ssq1[:], onesc, sq[:],
                                         start=(co == 0),
                                         stop=(co == NC_E - 1),
                                         skip_group_check=True)
                    x1ts = _ln_norm(nc, pbsb, pbbc, pbx1, h1ts, ssum1, ssq1,
                                    onesr_s, g1_c, be1_c, "x1")
                    # ---- FFN ----
                    hts = []
                    for j in range(NC_H):
                        hps = pbps.tile([128, TT], dt.float32, tag="mm")
                        for ci in range(NC_E):
                            nc.tensor.matmul(
                                hps[:],
                                w1_s[:, ci * HID + j * 128:
                                     ci * HID + (j + 1) * 128],
                                x1ts[ci][:],
                                start=(ci == 0), stop=(ci == NC_E - 1))
                        ht = pbh.tile([128, TT], dt.bfloat16, tag="ht")
                        nc.scalar.activation(ht[:], hps[:], AF.Relu,
                                             bias=b1_c(j))
                        hts.append(ht)
                    h2ts = []
                    ssum2 = pbst.tile([1, TT], dt.float32, tag="st1")
                    ssq2 = pbst.tile([1, TT], dt.float32, tag="st1")
                    for co in range(NC_E):
                        ops2 = pbps.tile([128, TT], dt.float32, tag="mm")
                        for j in range(NC_H):
                            nc.tensor.matmul(
                                ops2[:],
                                w2_s[:, j * E + co * 128:
                                     j * E + (co + 1) * 128],
                                hts[j][:],
                                start=(j == 0), stop=(j == NC_H - 1))
                        h2t = pbo.tile([128, TT], dt.bfloat16, tag="h2")
                        nc.vector.scalar_tensor_tensor(
                            out=h2t[:], in0=ops2[:], scalar=b2_c(co),
                            in1=x1ts[co][:], op0=ALU.add, op1=ALU.add)
                        h2ts.append(h2t)
                        sq = pbsb.tile([128, TT], dt.bfloat16, tag="sq")
                        nc.vector.tensor_mul(sq[:], h2t[:], h2t[:])
                        nc.tensor.matmul(ssum2[:], onesc, h2t[:],
                                         start=(co == 0),
                                         stop=(co == NC_E - 1),
                                         skip_group_check=True)
                        nc.tensor.matmul(ssq2[:], onesc, sq[:],
                                         start=(co == 0),
                                         stop=(co == NC_E - 1),
                                         skip_group_check=True)
                    outs = _ln_norm(nc, pbsb, pbbc, pbo, h2ts, ssum2, ssq2,
                                    onesr_s, g2_c, be2_c, "ou")
                    # ---- transpose back to token-major, DMA out ----
                    for j in range(NJ):
                        otok = pbsb.tile([128, E], dt.int8, tag="otok")
                        for c in range(NC_E):
                            ps = tpps2.tile([128, 128], dt.bfloat16, tag="tp2")
                            nc.tensor.transpose(
                                ps[:], outs[c][:, j * 128:(j + 1) * 128],
                                idb)
                            nc.vector.tensor_scalar_mul(
                                otok[:, c * 128:(c + 1) * 128], ps[:], OSCALE)
                        nc.sync.dma_start(
                            out=out_d[t0 + j * 128: t0 + (j + 1) * 128, :],
                            in_=otok[:])

    nc.compile()
    return nc


def _prep_in_maps(inputs):
    x = np.asarray(inputs["x"], dtype=F32)
    blob8 = _CACHE.get("blob8")
    if blob8 is None:
        blob8 = _CACHE["blob8"] = np.zeros((NCORES, BLOB_ROWS, 512),
                                           np.uint8)
        _CACHE["xq_tmp"] = np.empty((B, S, E), np.float32)
    tmp = _CACHE["xq_tmp"]

    # x -> int8 rows 0:4096 (tmp holds exact integers, copyto truncates)
    np.multiply(x, XSCALE, out=tmp)
    np.rint(tmp, out=tmp)
    np.clip(tmp, -127, 127, out=tmp)
    np.copyto(blob8[:, :S, :].view(np.int8), tmp, casting="unsafe")

    # weight shards
    wa = np.concatenate([
        np.asarray(inputs["Wq"], F32).astype(BF16),
        np.asarray(inputs["Wk"], F32).astype(BF16),
        np.asarray(inputs["Wv"], F32).astype(BF16),
        np.asarray(inputs["Wo"], F32).astype(BF16),
        np.asarray(inputs["W2"], F32).astype(BF16),
    ], axis=0)                                     # [4*E+HID, E]
    wbb = np.asarray(inputs["W1"], F32).astype(BF16)   # [E, HID]
    for b in range(NCORES):
        blob8[b, WAOFF:WBOFF] = \
            wa[b * 512:(b + 1) * 512].view(np.uint8).reshape(1024, 512)
        blob8[b, WBOFF:AUXOFF] = \
            wbb[b * 64:(b + 1) * 64].view(np.uint8).reshape(512, 512)

    # aux page [128, 256] bf16
    aux = np.zeros((128, 256), dtype=BF16)
    aux[:, 0:128] = np.eye(128, dtype=np.float32).astype(BF16)
    aux[0:64, 128] = BF16(1.0)
    aux[64:128, 129] = BF16(1.0)
    aux[:, 130] = BF16(1.0)
    aux[:, 131:135] = np.asarray(inputs["bk"], F32).astype(BF16) \
        .reshape(4, 128).T
    aux[:, 135:139] = np.asarray(inputs["bv"], F32).astype(BF16) \
        .reshape(4, 128).T
    pp = np.zeros((128, 44), dtype=F32)
    for c in range(4):
        pp[:, c] = inputs["bq"][c * 128:(c + 1) * 128]
        pp[:, 4 + c] = inputs["bo"][c * 128:(c + 1) * 128]
        pp[:, 24 + c] = inputs["b2"][c * 128:(c + 1) * 128]
        pp[:, 28 + c] = inputs["g1"][c * 128:(c + 1) * 128]
        pp[:, 32 + c] = inputs["be1"][c * 128:(c + 1) * 128]
        pp[:, 36 + c] = inputs["g2"][c * 128:(c + 1) * 128]
        pp[:, 40 + c] = inputs["be2"][c * 128:(c + 1) * 128]
    for j in range(16):
        pp[:, 8 + j] = inputs["b1"][j * 128:(j + 1) * 128]
    hi = pp.astype(BF16)
    aux[:, 139:183] = hi
    aux[:, 183:227] = (pp - hi.astype(F32)).astype(BF16)
    blob8[:, AUXOFF:] = aux.view(np.uint8)[None]

    return [{"blob": blob8[b]} for b in range(NCORES)]


def kernel(**inputs):
    if "nc" not in _CACHE:
        import jax
        try:
            import tempfile
            jax.config.update("jax_compilation_cache_dir",
                              tempfile.gettempdir() + "/jaxcache_bass")
            jax.config.update("jax_persistent_cache_min_compile_time_secs",
                              0.0)
            jax.config.update("jax_persistent_cache_min_entry_size_bytes", 0)
        except Exception:
            pass
        _CACHE["nc"] = _build()
    nc = _CACHE["nc"]
    in_maps = _prep_in_maps(inputs)

    res = run_bass_kernel_spmd(nc, in_maps, core_ids=list(range(NCORES)),
                               **_CACHE.get("run_kwargs", {}))
    _CACHE["last"] = res
    outf = _CACHE.get("outf")
    if outf is None:
        outf = _CACHE["outf"] = np.empty((B, S, E), F32)
    for b in range(NCORES):
        np.multiply(res.results[b]["out"], F32(1.0 / OSCALE),
                    dtype=F32, out=outf[b], casting="unsafe")
    return outf

